# revision 26
# baseline (speedup 1.0000x reference)
"""NLinear (per-feature grouped linear) Trainium2 Bass kernel, 8-core SPMD.

Problem: x [4096, 64, 256] f32, weight [64, 256, 256] f32, b [64, 256] f32
         out[b,f,:] = x[b,f,:] @ weight[f] + b[f]

Strategy:
  - Shard the 64 features across 8 NeuronCores (8 features per core) --
    expert-style grouped GEMM; each core holds its features' weights.
  - The kernel is HBM-bound, so x, weight and the output travel as bf16 on
    the wire: host downcasts x/w, the device accumulates in f32 PSUM, adds
    the f32 bias on the DVE while converting the result to bf16, and the
    host upcasts the output back to f32. This halves HBM traffic
    (~69 MB -> ~35 MB per core).
  - Host packs x per (batch-strip, k-chunk) into contiguous DRAM blocks
    [128, f*strip] (ff-major, batch-minor columns) so strips load with a
    few big DMAs whose per-partition descriptor is 4 KB (near line-rate).
    Weights are host-packed to a single [128, f*nk*o] tile (one 1 MB DMA,
    8 KB rows). No on-chip transposes.
  - Graduated strip schedule (512, 512, 1024, 1024, 1024 batches): the
    first matmul block is gated on only 2.1 MB of loads, shrinking the
    pipeline fill, while later strips use maximal DMAs.
  - x loads issue on the sync-engine HWDGE ring; output stores issue on the
    scalar-engine HWDGE ring so a store waiting on its bias-add never
    stalls upcoming loads (separate FIFOs). Big-strip tiles are triple
    buffered so loads run a full strip ahead.
  - Per 128-batch subtile, 16 matmuls (stationary = x slice, moving =
    weight chunk [128, 256]) accumulate into one wide [128, 2048] PSUM
    tile (4 banks, double buffered); one wide DVE tensor_add fuses the
    bias-add with the PSUM->SBUF copy + bf16 convert; one 512 KB DMA
    stores each output tile in natural layout.
"""

import sys

sys.path.insert(0, "/opt/trn_rl_repo")

import numpy as np

_STATE = {}

B, F, K, O = 4096, 64, 256, 256
NCORES = 8
FL = F // NCORES
NK = K // 128
SMALL, BIG = 512, 1024
STRIPS = (SMALL, SMALL, BIG, BIG, BIG)  # sums to B
NSMALL = STRIPS.count(SMALL)
NBIG = STRIPS.count(BIG)


def _build_nc():
    import concourse.bacc as bacc
    import concourse.bass as bass
    import concourse.mybir as mybir
    import concourse.tile as tile

    F32 = mybir.dt.float32
    BF16 = mybir.dt.bfloat16
    PSUM = bass.MemorySpace.PSUM

    f, k, o, nk = FL, K, O, NK

    nc = bacc.Bacc("TRN2", target_bir_lowering=False, debug=False)

    # x packed on host per strip: partition row p of block (s, c) holds
    # x[b, ff, c*128+p] for ff-major, batch-minor columns.
    xs_d = nc.dram_tensor(
        "xs", [NSMALL, nk, 128, f * SMALL], BF16, kind="ExternalInput"
    )
    xb_d = nc.dram_tensor(
        "xb", [NBIG, nk, 128, f * BIG], BF16, kind="ExternalInput"
    )
    wp_d = nc.dram_tensor("wp", [128, f * nk * o], BF16, kind="ExternalInput")
    brow_d = nc.dram_tensor(
        "b_row", [1, (f - 2) * o], F32, kind="ExternalInput"
    )
    o_d = nc.dram_tensor("o", [B, f * o], BF16, kind="ExternalOutput")

    with tile.TileContext(nc) as tc:
        with (
            tc.tile_pool(name="wpool", bufs=1) as wpool,
            tc.tile_pool(name="const", bufs=1) as const,
            tc.tile_pool(name="xsp", bufs=2) as xsp,
            tc.tile_pool(name="xbp", bufs=3) as xbp,
            tc.tile_pool(name="opool", bufs=6) as opool,
            tc.tile_pool(name="pso", bufs=2, space=PSUM) as pso,
        ):
            # DVE evacuates PSUM banks 0-2 (features 0..5) with the bias
            # fused; ScalarE (ACT) pure-copies bank 3 (features 6..7) in
            # parallel -- their bias is added host-side after upcast.
            vh = (f - 2) * o
            w_all = wpool.tile([128, f * nk * o], BF16)
            nc.scalar.dma_start(w_all[:], wp_d.ap())
            # Bias lands as one row; gpsimd broadcasts it across partitions
            # (saves a 1 MB replicated HBM load in the fill).
            brow = const.tile([1, vh], F32)
            bias_bc = const.tile([128, vh], F32)
            nc.sync.dma_start(brow[:], brow_d.ap())
            nc.gpsimd.partition_broadcast(bias_bc[:], brow[:])

            batch0 = 0
            small_i = big_i = 0
            pending = None  # (o_t, row) store delayed by one tile
            for strip in STRIPS:
                if strip == SMALL:
                    pool, tagp, src, si = xsp, "xs", xs_d, small_i
                    small_i += 1
                else:
                    pool, tagp, src, si = xbp, "xb", xb_d, big_i
                    big_i += 1
                # Chunked DMAs (4 KB/partition rows) rather than one big
                # one: the ~8-deep DMA in-flight window then paces loads so
                # earlier strips complete first (queued mega-DMAs round-
                # robin at packet granularity and all finish together,
                # starving the first strip).
                q = 2048
                xc = []
                for c in range(nk):
                    xtile = pool.tile([128, f * strip], BF16, tag=f"{tagp}_{c}")
                    for h in range((f * strip) // q):
                        nc.sync.dma_start(
                            xtile[:, h * q : (h + 1) * q],
                            src.ap()[si, c, :, h * q : (h + 1) * q],
                        )
                    xc.append(xtile)
                for j in range(strip // 128):
                    o_t = opool.tile([128, f * o], BF16)
                    # One wide PSUM tile (4 banks) holds all 8 features'
                    # accumulators; a single wide DVE add amortizes the
                    # per-instruction overhead (151 cyc) 8x.
                    po = pso.tile([128, f * o], F32, tag="po")
                    for ff in range(f):
                        for c in range(nk):
                            nc.tensor.matmul(
                                po[:, ff * o : (ff + 1) * o],
                                xc[c][
                                    :,
                                    ff * strip + j * 128 : ff * strip + (j + 1) * 128,
                                ],
                                w_all[:, (ff * nk + c) * o : (ff * nk + c + 1) * o],
                                start=(c == 0),
                                stop=(c == nk - 1),
                            )
                    # Parallel PSUM drain on disjoint banks; the store for
                    # the PREVIOUS tile issues after this tile's copy so
                    # its (long-done) dependencies never head-of-line
                    # block the ACT ring.
                    nc.vector.tensor_add(o_t[:, 0:vh], po[:, 0:vh], bias_bc[:])
                    nc.scalar.copy(o_t[:, vh:], po[:, vh:])
                    if pending is not None:
                        p_t, p_row = pending
                        nc.scalar.dma_start(
                            o_d.ap()[p_row : p_row + 128, :], p_t[:]
                        )
                    pending = (o_t, batch0 + j * 128)
                batch0 += strip
            p_t, p_row = pending
            nc.scalar.dma_start(o_d.ap()[p_row : p_row + 128, :], p_t[:])

    nc.compile()
    return nc


def _pack_x(xc_bf):
    """Per-core x [B, f, K] bf16 -> (xs [NSMALL,nk,128,f*SMALL],
    xb [NBIG,nk,128,f*BIG]) with ff-major, batch-minor columns."""
    blocks = []
    b0 = 0
    for strip in STRIPS:
        blk = xc_bf[b0 : b0 + strip]  # [strip, f, K]
        blocks.append(
            np.ascontiguousarray(
                blk.reshape(strip, FL, NK, 128).transpose(2, 3, 1, 0)
            ).reshape(NK, 128, FL * strip)
        )
        b0 += strip
    xs = np.stack(blocks[:NSMALL])
    xb = np.stack(blocks[NSMALL:])
    return xs, xb


def _shard_inputs(x, weight, b):
    """Full f32 inputs -> per-core input maps."""
    import ml_dtypes

    bf16 = ml_dtypes.bfloat16
    x_bf = x.astype(bf16)  # [B, F, K]
    w_bf = weight.astype(bf16)
    maps = []
    for c in range(NCORES):
        xs, xb = _pack_x(x_bf[:, c * FL : (c + 1) * FL, :])
        wc = w_bf[c * FL : (c + 1) * FL]  # [f, K, O]
        wp = np.ascontiguousarray(
            wc.reshape(FL, NK, 128, O).transpose(2, 0, 1, 3)
        ).reshape(128, FL * NK * O)
        maps.append(
            {
                "xs": xs,
                "xb": xb,
                "wp": wp,
                "b_row": np.ascontiguousarray(
                    b[c * FL : c * FL + FL - 2].reshape(1, (FL - 2) * O)
                ),
            }
        )
    return maps


def _unshard_outputs(results, b):
    """Per-core result maps (bf16) -> full f32 [B, F, O] output. The device
    only added bias for features 0..FL-3 of each core (DVE half); add the
    last two features' bias here in f32."""
    out = np.empty((B, F, O), np.float32)
    for c, rm in enumerate(results):
        blk = np.asarray(rm["o"]).astype(np.float32).reshape(B, FL, O)
        blk[:, FL - 2 :, :] += b[c * FL + FL - 2 : (c + 1) * FL][None]
        out[:, c * FL : (c + 1) * FL, :] = blk
    return out


def kernel(x: np.ndarray, weight: np.ndarray, b: np.ndarray) -> np.ndarray:
    assert x.shape == (B, F, K) and weight.shape == (F, K, O) and b.shape == (F, O)
    x = np.ascontiguousarray(x, dtype=np.float32)
    weight = np.ascontiguousarray(weight, dtype=np.float32)
    b = np.ascontiguousarray(b, dtype=np.float32)

    from concourse import bass2jax

    if "nc" not in _STATE:
        _STATE["nc"] = _build_nc()
    results = bass2jax.run_bass_via_pjrt(
        _STATE["nc"], _shard_inputs(x, weight, b), n_cores=NCORES
    )
    return _unshard_outputs(results, b)


if __name__ == "__main__":
    rng = np.random.default_rng(0)
    x = rng.standard_normal((B, F, K), dtype=np.float32)
    w = (rng.uniform(-1, 1, (F, K, O)) / 16).astype(np.float32)
    bias = (rng.uniform(-1, 1, (F, O)) / 16).astype(np.float32)
    out = kernel(x=x, weight=w, b=bias)
    ref = np.einsum("bfk,fko->bfo", x, w) + bias[None]
    err = np.abs(out - ref).max() / np.abs(ref).max()
    print("self-test relerr:", err)
